# revision 5
# baseline (speedup 1.0000x reference)
"""LIF neuron with soft reset — Trainium2 Bass kernel, 8-way data parallel.

Problem: x (T=32, B=16, C=128, H=32, W=32) f32. Sequential scan over T:
    mem = 0.25*mem + x[t];  s[t] = (mem >= 1);  mem -= s[t]
Returns spikes (same shape, f32 values in {0,1}).

Sharding: batch dim B=16 split 2-per-core across 8 NeuronCores; the scan is
elementwise over (B,C,H,W) so cores are fully independent. Per-core slice of
one timestep = 2*128*32*32 = 262144 contiguous floats -> [128 x 2048] f32.

Scaled-threshold formulation: carry PRE-reset state P_t = 4^t * mem_pre_t,
with y_t = 4^t * x_t pre-scaled on the host (exact power-of-two multiply).
    P_t   = (P_{t-1} - 4^{t-1} * [P_{t-1} >= 4^{t-1}]) + y_t
    s_t   = [P_t >= 4^t]
Power-of-two rescaling commutes with IEEE-754 rounding; the reset subtract
is exact; the only rounding per step is the +y_t add -- so every P_t is
bit-identical to the reference fp32 sequence.

v7 engine assignment (v6 + tail pipelining; per-core DMA is the wall at
~358 GB/s HBM-per-NC, so all that's left is the ramp/tail):
    DVE    : custom fused op LIF_STEP_ANT per step (the serial recurrence),
             out = (Src0 - C0*(Src0 > C1)) + Src1, over 6 rotating state
             tiles. Steps 28-31 are split into 4x512-column chunks so the
             dependent chain pipelines against the last loads. DVE also
             copies group 3's PSUM out (x512 scaled) in its tail gaps.
    ACT    : Sign(P - prevfloat(4^t)) -> u8 spike in {0,1} (fp32->u8
             conversion saturates -1 to 0; equality -> 0, matching >=
             exactly). Copies groups 0-2's packed PSUM out with scale=512.
    PE     : packs 8 (7 for group 3) steps/byte: spike u8 bitcast fp8e4
             (0x01 = 2^-9 denormal), matmul with diag(2^k) fp8 stationary,
             PSUM fp32 accumulate (exact: 2^-9 * 2^k * 512 = 2^k).
    DMA    : x in flat [P, T*F] layout (contiguous per-partition rows ->
             large clean descriptors); 4 MB batches for steps 0-27, then
             1 MB per step for 28-30 and 4x256 KB for 31, so the serial
             tail starts as each step's bytes land instead of waiting for
             a 4 MB batch semaphore. Loads ride the SP HWDGE ring; all
             stores ride the ACT ring. Output is bit-packed (groups 0-2:
             steps 8g..8g+7; group 3: steps 24..30 in bits 0-6; step 31
             raw u8 in its own tensor, stored chunk-wise as ACT signs it,
             so no full-width pack copy sits on the tail).
"""

import numpy as np

T, B, C, H, W = 32, 16, 128, 32, 32
NCORES = 8
BPC = B // NCORES          # batches per core
P = 128                    # SBUF partitions
F = (BPC * C * H * W) // P # 2048 free-dim elements per step
Q = 512                    # tail chunk columns (= PSUM bank)
NSTATE = 6                 # rotating DVE state tiles
G8 = T // 8                # packed output groups
CHUNK_T0 = 28              # first column-chunked step

_cache = {}


def _prevfloat(v: float) -> float:
    return float(np.nextafter(np.float32(v), np.float32(0)))


def _register_lif_op():
    """Append the fused LIF step op to the custom-DVE registry (idempotent).

    out = (Src0 - s0 * (Src0 > s1)) + Src1
    s0 = 4^(t-1) reset amount, s1 = prevfloat(4^(t-1)) so the strict > equals
    the reference's >= on fp32 values.
    """
    from concourse import dve_ops
    from concourse.dve_spec import Spec, Src0, Src1, C0, C1, lower, _has_src1
    from concourse.dve_uop import DveOpSpec

    for op in dve_ops.OPS:
        if op.name == "LIF_STEP_ANT":
            return op

    spec = Spec(
        body=(Src0 - C0 * (Src0 > C1)) + Src1,
        reference=lambda in0, in1, s0, s1, imm2: (
            in0 - s0 * (in0 > s1).astype(np.float32)
        )
        + in1,
    )
    op = dve_ops.DveOp("LIF_STEP_ANT", spec, subdim=False, uops_sha={})
    dve_ops.OPS.append(op)
    dve_ops.CUSTOM_DVE_SPECS[op.name] = op.spec
    dve_ops._SUB_OPCODE_FOR_NAME[op.name] = (
        dve_ops._CUSTOM_DVE_ROW_BASE + len(dve_ops.OPS) - 1
    )
    # self-pin the sha exactly the way DveOp.compile() derives it
    for ver in ("v3", "v4"):
        try:
            compiled = DveOpSpec(
                name=op.name,
                opcode=dve_ops.get_dve_sub_opcode(op.name),
                uops=lower(spec, ver=ver),
                rd1_en=_has_src1(spec),
            )
            op.uops_sha[ver] = compiled.sha(ver)
        except Exception:
            pass
    return op


# load schedule: step -> number of steps in the batch issued at that step
LOADS = {0: 4, 4: 4, 8: 4, 12: 4, 16: 4, 20: 4, 24: 4, 28: 1, 29: 1, 30: 1, 31: 1}


def _build(reps: int = 1, timing: bool = False):
    import concourse.bacc as bacc
    import concourse.mybir as mybir
    from concourse.bass import MemorySpace
    from concourse.tile import TileContext

    kin = "Internal" if timing else "ExternalInput"
    kout = "Internal" if timing else "ExternalOutput"

    nc = bacc.Bacc(None, target_bir_lowering=False)
    x_d = nc.dram_tensor("x", [P, T * F], mybir.dt.float32, kind=kin)
    w_d = nc.dram_tensor("w", [P, 8 * 128], mybir.dt.float8e4, kind=kin)
    th_d = nc.dram_tensor("th", [P, T], mybir.dt.float32, kind=kin)
    o_d = nc.dram_tensor("o", [G8, P, F], mybir.dt.uint8, kind=kout)
    o31_d = nc.dram_tensor("o31", [P, F], mybir.dt.uint8, kind=kout)
    if timing:
        # tiny anchor I/O so the timing NEFF ships ~nothing over axon
        din = nc.dram_tensor("din", [1, 64], mybir.dt.int32, kind="ExternalInput")
        dout = nc.dram_tensor("dout", [1, 64], mybir.dt.int32, kind="ExternalOutput")

    fp32 = mybir.dt.float32
    u8 = mybir.dt.uint8
    fp8 = mybir.dt.float8e4
    Act = mybir.ActivationFunctionType
    lif = _register_lif_op()

    with TileContext(nc) as tc:
        with (
            tc.tile_pool(name="mem", bufs=1) as mempool,
            tc.tile_pool(name="xin", bufs=4) as xpool,
            tc.tile_pool(name="spk", bufs=6) as spool,
            tc.tile_pool(name="pk", bufs=2) as pkpool,
            tc.tile_pool(name="wp", bufs=1) as wpool,
            tc.tile_pool(name="ps", bufs=2, space=MemorySpace.PSUM) as pspool,
        ):
            if timing:
                dtile = wpool.tile([1, 64], mybir.dt.int32, name="dt", tag="dt")
                nc.scalar.dma_start(out=dtile, in_=din[:, :])
                nc.scalar.dma_start(out=dout[:, :], in_=dtile)
            # consts ride the ACT ring so the first x load owns the SP ring
            wt = wpool.tile([P, 8 * 128], fp8, name="w", tag="w")
            nc.scalar.dma_start(out=wt, in_=w_d[:, :])
            th = wpool.tile([P, T], fp32, name="th", tag="th")
            nc.scalar.dma_start(out=th, in_=th_d[:, :])
            pst = [
                mempool.tile([P, F], fp32, name=f"p{i}", tag=f"p{i}")
                for i in range(NSTATE)
            ]
            for _ in range(reps):  # reps>1 only for benchmarking
                acc = None
                for t in range(T):
                    g, k = divmod(t, 8)
                    if t in LOADS:
                        n = LOADS[t]
                        xt = xpool.tile([P, 4 * F], fp32, name="x", tag="x")
                        xoff = t
                        if t == T - 1:
                            # last step: 4 quarter-loads so the tail chain
                            # starts as soon as each 256 KB lands
                            for c0 in range(0, F, Q):
                                nc.sync.dma_start(
                                    out=xt[:, c0 : c0 + Q],
                                    in_=x_d[:, t * F + c0 : t * F + c0 + Q],
                                )
                        else:
                            nc.sync.dma_start(
                                out=xt[:, : n * F],
                                in_=x_d[:, t * F : (t + n) * F],
                            )
                    if k == 0:
                        # per-chunk PSUM tiles (1 bank each) so readers of a
                        # finished column chunk don't wait on other chunks'
                        # matmuls (PSUM deps are tracked per tile)
                        acc = [
                            pspool.tile([P, Q], fp32, name=f"acc{c}", tag=f"acc{c}")
                            for c in range(F // Q)
                        ]
                    # reset uses the PREVIOUS step's threshold; t=0 resets
                    # nothing (state is 0), s0=0 makes the op a plain add
                    rst = float(4.0 ** (t - 1)) if t > 0 else 0.0
                    rthr = _prevfloat(4.0 ** (t - 1)) if t > 0 else 1.0
                    src = pst[t % NSTATE]
                    dst = pst[(t + 1) % NSTATE]
                    ycol = (t - xoff) * F
                    kstop = 7 if g < G8 - 1 else 6
                    st = spool.tile([P, F], u8, name="s", tag="s")
                    s8 = st.bitcast(fp8)

                    if t < CHUNK_T0:
                        yt = xt[:, ycol : ycol + F]
                        if t == 0:
                            # P_0 = y_0 exactly: a 2x-mode copy replaces the
                            # state memset + full custom op
                            nc.vector.tensor_copy(dst, yt)
                        else:
                            nc.vector._custom_dve(
                                lif, out=dst, in0=src, in1=yt, s0=rst, s1=rthr
                            )
                        nc.scalar.activation(
                            st, dst, Act.Sign, bias=th[:, t : t + 1]
                        )
                        for c0 in range(0, F, Q):
                            nc.tensor.matmul(
                                acc[c0 // Q],
                                wt[:, 128 * k : 128 * (k + 1)],
                                s8[:, c0 : c0 + Q],
                                start=(k == 0),
                                stop=(k == kstop),
                            )
                        if k == 7 and g < G8 - 1:
                            pk = pkpool.tile([P, F], u8, name="pk", tag="pk")
                            for c0 in range(0, F, Q):
                                nc.scalar.activation(
                                    pk[:, c0 : c0 + Q], acc[c0 // Q],
                                    Act.Copy, scale=512.0,
                                )
                            nc.scalar.dma_start(out=o_d[g], in_=pk)
                    elif t < T - 1:
                        # steps 28-30: column-chunked recurrence + sign + mm
                        for c0 in range(0, F, Q):
                            sl = slice(c0, c0 + Q)
                            nc.vector._custom_dve(
                                lif,
                                out=dst[:, sl],
                                in0=src[:, sl],
                                in1=xt[:, ycol + c0 : ycol + c0 + Q],
                                s0=rst,
                                s1=rthr,
                            )
                            nc.scalar.activation(
                                st[:, sl], dst[:, sl], Act.Sign,
                                bias=th[:, t : t + 1],
                            )
                            nc.tensor.matmul(
                                acc[c0 // Q],
                                wt[:, 128 * k : 128 * (k + 1)],
                                s8[:, sl],
                                start=(k == 0),
                                stop=(k == kstop),
                            )
                        if t == T - 2:
                            # group 3's PSUM is final after step 30 (k=6);
                            # copy it out on DVE (scaled x512 -> integer u8)
                            # while DVE waits for step 31's quarter-loads.
                            # high_priority so the scheduler doesn't sink
                            # these behind step 31's recurrence chunks.
                            pk = pkpool.tile([P, F], u8, name="pk", tag="pk")
                            with tc.high_priority():
                                for c0 in range(0, F, Q):
                                    sl = slice(c0, c0 + Q)
                                    nc.vector.tensor_scalar_mul(
                                        pk[:, sl], acc[c0 // Q], 512.0
                                    )
                                    if c0 == Q:
                                        nc.scalar.dma_start(
                                            out=o_d[G8 - 1][:, : 2 * Q],
                                            in_=pk[:, : 2 * Q],
                                        )
                                    elif c0 == 3 * Q:
                                        nc.scalar.dma_start(
                                            out=o_d[G8 - 1][:, 2 * Q :],
                                            in_=pk[:, 2 * Q :],
                                        )
                    else:
                        # step 31: chunked recurrence + sign -> raw u8 store
                        for c0 in range(0, F, Q):
                            sl = slice(c0, c0 + Q)
                            nc.vector._custom_dve(
                                lif,
                                out=dst[:, sl],
                                in0=src[:, sl],
                                in1=xt[:, ycol + c0 : ycol + c0 + Q],
                                s0=rst,
                                s1=rthr,
                            )
                            nc.scalar.activation(
                                st[:, sl], dst[:, sl], Act.Sign,
                                bias=th[:, t : t + 1],
                            )
                            nc.scalar.dma_start(
                                out=o31_d[:, sl], in_=st[:, sl]
                            )
    nc.finalize()
    return nc


def _consts():
    import ml_dtypes

    w = np.zeros((P, 8 * 128), ml_dtypes.float8_e4m3)
    eye = np.eye(128)
    for k in range(8):
        w[:, 128 * k : 128 * (k + 1)] = (2.0**k * eye).astype(ml_dtypes.float8_e4m3)
    th = np.empty((P, T), np.float32)
    for t in range(T):
        th[:, t] = -np.float32(_prevfloat(4.0**t))
    return w, th


def kernel(x: np.ndarray) -> np.ndarray:
    from concourse.bass_utils import run_bass_kernel_spmd

    assert x.shape == (T, B, C, H, W) and x.dtype == np.float32
    if "nc" not in _cache:
        _cache["nc"] = _build()
    nc = _cache["nc"]

    # host-side pre-scale: y_t = 4^t * x_t (exact power-of-two multiply)
    scale = (4.0 ** np.arange(T, dtype=np.float64)).astype(np.float32)
    y = x * scale[:, None, None, None, None]
    w, th = _consts()

    in_maps = []
    for c in range(NCORES):
        yk = (
            y[:, c * BPC : (c + 1) * BPC]
            .reshape(T, P, F)
            .transpose(1, 0, 2)
            .reshape(P, T * F)
        )
        in_maps.append({"x": np.ascontiguousarray(yk), "w": w, "th": th})

    res = run_bass_kernel_spmd(nc, in_maps, core_ids=list(range(NCORES)))
    _cache["last_result"] = res

    # unpack: group g byte bit k (LSB-first) = spike at t = 8g+k for g<3;
    # group 3 bits 0-6 = steps 24-30; step 31 is raw u8 in o31
    bits = np.arange(8, dtype=np.uint8)
    out = np.empty((T, B, C, H, W), dtype=np.float32)
    for c in range(NCORES):
        pk = res.results[c]["o"]  # [G8, P, F] u8
        o31 = res.results[c]["o31"]  # [P, F] u8
        sp = (pk[:, None] >> bits[None, :, None, None]) & np.uint8(1)
        sp[G8 - 1, 7] = o31
        out[:, c * BPC : (c + 1) * BPC] = (
            sp.reshape(T, BPC, C, H, W).astype(np.float32)
        )
    return out


# revision 6
# speedup vs baseline: 1.8014x; 1.8014x over previous
"""LIF neuron with soft reset — Trainium2 Bass kernel, 8-way data parallel.

Problem: x (T=32, B=16, C=128, H=32, W=32) f32. Sequential scan over T:
    mem = 0.25*mem + x[t];  s[t] = (mem >= 1);  mem -= s[t]
Returns spikes (same shape, f32 values in {0,1}).

Sharding: batch dim B=16 split 2-per-core across 8 NeuronCores; the scan is
elementwise over (B,C,H,W) so cores are fully independent. Per-core slice of
one timestep = 2*128*32*32 = 262144 contiguous floats -> [128 x 2048] f32.

Scaled-threshold formulation: carry PRE-reset state P_t = 4^t * mem_pre_t,
with y_t = 4^t * x_t pre-scaled on the host (exact power-of-two multiply).
    P_t   = (P_{t-1} - 4^{t-1} * [P_{t-1} >= 4^{t-1}]) + y_t
    s_t   = [P_t >= 4^t]
Power-of-two rescaling commutes with IEEE-754 rounding; the reset subtract
is exact; the only rounding per step is the +y_t add -- so every P_t is
bit-identical to the reference fp32 sequence.

v7 engine assignment (v6 + tail pipelining; per-core DMA is the wall at
~358 GB/s HBM-per-NC, so all that's left is the ramp/tail):
    DVE    : custom fused op LIF_STEP_ANT per step (the serial recurrence),
             out = (Src0 - C0*(Src0 > C1)) + Src1, over 6 rotating state
             tiles. Steps 28-31 are split into 4x512-column chunks so the
             dependent chain pipelines against the last loads. DVE also
             copies group 3's PSUM out (x512 scaled) in its tail gaps.
    ACT    : Sign(P - prevfloat(4^t)) -> u8 spike in {0,1} (fp32->u8
             conversion saturates -1 to 0; equality -> 0, matching >=
             exactly). Copies groups 0-2's packed PSUM out with scale=512.
    PE     : packs 8 (7 for group 3) steps/byte: spike u8 bitcast fp8e4
             (0x01 = 2^-9 denormal), matmul with diag(2^k) fp8 stationary,
             PSUM fp32 accumulate (exact: 2^-9 * 2^k * 512 = 2^k).
    DMA    : x in flat [P, T*F] layout (contiguous per-partition rows ->
             large clean descriptors); 4 MB batches for steps 0-27, then
             1 MB per step for 28-30 and 4x256 KB for 31, so the serial
             tail starts as each step's bytes land instead of waiting for
             a 4 MB batch semaphore. Loads ride the SP HWDGE ring; all
             stores ride the ACT ring. Output is bit-packed (groups 0-2:
             steps 8g..8g+7; group 3: steps 24..30 in bits 0-6; step 31
             raw u8 in its own tensor, stored chunk-wise as ACT signs it,
             so no full-width pack copy sits on the tail).
"""

import numpy as np

T, B, C, H, W = 32, 16, 128, 32, 32
NCORES = 8
BPC = B // NCORES          # batches per core
P = 128                    # SBUF partitions
F = (BPC * C * H * W) // P # 2048 free-dim elements per step
Q = 512                    # tail chunk columns (= PSUM bank)
NSTATE = 6                 # rotating DVE state tiles
G8 = T // 8                # packed output groups
CHUNK_T0 = 28              # first column-chunked step

_cache = {}


def _prevfloat(v: float) -> float:
    return float(np.nextafter(np.float32(v), np.float32(0)))


def _register_lif_op():
    """Append the fused LIF step op to the custom-DVE registry (idempotent).

    out = (Src0 - s0 * (Src0 > s1)) + Src1
    s0 = 4^(t-1) reset amount, s1 = prevfloat(4^(t-1)) so the strict > equals
    the reference's >= on fp32 values.
    """
    from concourse import dve_ops
    from concourse.dve_spec import Spec, Src0, Src1, C0, C1, lower, _has_src1
    from concourse.dve_uop import DveOpSpec

    for op in dve_ops.OPS:
        if op.name == "LIF_STEP_ANT":
            return op

    spec = Spec(
        body=(Src0 - C0 * (Src0 > C1)) + Src1,
        reference=lambda in0, in1, s0, s1, imm2: (
            in0 - s0 * (in0 > s1).astype(np.float32)
        )
        + in1,
    )
    op = dve_ops.DveOp("LIF_STEP_ANT", spec, subdim=False, uops_sha={})
    dve_ops.OPS.append(op)
    dve_ops.CUSTOM_DVE_SPECS[op.name] = op.spec
    dve_ops._SUB_OPCODE_FOR_NAME[op.name] = (
        dve_ops._CUSTOM_DVE_ROW_BASE + len(dve_ops.OPS) - 1
    )
    # self-pin the sha exactly the way DveOp.compile() derives it
    for ver in ("v3", "v4"):
        try:
            compiled = DveOpSpec(
                name=op.name,
                opcode=dve_ops.get_dve_sub_opcode(op.name),
                uops=lower(spec, ver=ver),
                rd1_en=_has_src1(spec),
            )
            op.uops_sha[ver] = compiled.sha(ver)
        except Exception:
            pass
    return op


# load schedule: step -> number of steps in the batch issued at that step
LOADS = {0: 4, 4: 4, 8: 4, 12: 4, 16: 4, 20: 4, 24: 4, 28: 1, 29: 1, 30: 1, 31: 1}


def _build(reps: int = 1, timing: bool = False, rep_barrier: bool = False):
    import concourse.bacc as bacc
    import concourse.mybir as mybir
    from concourse.bass import MemorySpace
    from concourse.tile import TileContext

    kin = "Internal" if timing else "ExternalInput"
    kout = "Internal" if timing else "ExternalOutput"

    nc = bacc.Bacc(None, target_bir_lowering=False)
    x_d = nc.dram_tensor("x", [P, T * F], mybir.dt.float32, kind=kin)
    w_d = nc.dram_tensor("w", [P, 8 * 128], mybir.dt.float8e4, kind=kin)
    th_d = nc.dram_tensor("th", [P, T], mybir.dt.float32, kind=kin)
    o_d = nc.dram_tensor("o", [G8, P, F], mybir.dt.uint8, kind=kout)
    o31_d = nc.dram_tensor("o31", [P, F], mybir.dt.uint8, kind=kout)
    if timing:
        # tiny anchor I/O so the timing NEFF ships ~nothing over axon
        din = nc.dram_tensor("din", [1, 64], mybir.dt.int32, kind="ExternalInput")
        dout = nc.dram_tensor("dout", [1, 64], mybir.dt.int32, kind="ExternalOutput")

    fp32 = mybir.dt.float32
    u8 = mybir.dt.uint8
    fp8 = mybir.dt.float8e4
    Act = mybir.ActivationFunctionType
    lif = _register_lif_op()

    with TileContext(nc) as tc:
        with (
            tc.tile_pool(name="mem", bufs=1) as mempool,
            tc.tile_pool(name="xin", bufs=4) as xpool,
            tc.tile_pool(name="spk", bufs=6) as spool,
            tc.tile_pool(name="pk", bufs=2) as pkpool,
            tc.tile_pool(name="wp", bufs=1) as wpool,
            tc.tile_pool(name="ps", bufs=2, space=MemorySpace.PSUM) as pspool,
        ):
            if timing:
                dtile = wpool.tile([1, 64], mybir.dt.int32, name="dt", tag="dt")
                nc.scalar.dma_start(out=dtile, in_=din[:, :])
                nc.scalar.dma_start(out=dout[:, :], in_=dtile)
            # consts ride the ACT ring so the first x load owns the SP ring
            wt = wpool.tile([P, 8 * 128], fp8, name="w", tag="w")
            nc.scalar.dma_start(out=wt, in_=w_d[:, :])
            th = wpool.tile([P, T], fp32, name="th", tag="th")
            nc.scalar.dma_start(out=th, in_=th_d[:, :])
            pst = [
                mempool.tile([P, F], fp32, name=f"p{i}", tag=f"p{i}")
                for i in range(NSTATE)
            ]
            for _rep in range(reps):  # reps>1 only for benchmarking
                if rep_barrier and _rep > 0:
                    # serialize reps so marginal-reps timing includes the
                    # ramp + tail of every rep (benchmarking only)
                    tc.strict_bb_all_engine_barrier()
                acc = None
                for t in range(T):
                    g, k = divmod(t, 8)
                    if t in LOADS:
                        n = LOADS[t]
                        xt = xpool.tile([P, 4 * F], fp32, name="x", tag="x")
                        xoff = t
                        if t == T - 1:
                            # last step: 4 quarter-loads so the tail chain
                            # starts as soon as each 256 KB lands
                            for c0 in range(0, F, Q):
                                nc.sync.dma_start(
                                    out=xt[:, c0 : c0 + Q],
                                    in_=x_d[:, t * F + c0 : t * F + c0 + Q],
                                )
                        else:
                            nc.sync.dma_start(
                                out=xt[:, : n * F],
                                in_=x_d[:, t * F : (t + n) * F],
                            )
                    if k == 0:
                        # per-chunk PSUM tiles (1 bank each) so readers of a
                        # finished column chunk don't wait on other chunks'
                        # matmuls (PSUM deps are tracked per tile)
                        acc = [
                            pspool.tile([P, Q], fp32, name=f"acc{c}", tag=f"acc{c}")
                            for c in range(F // Q)
                        ]
                    # reset uses the PREVIOUS step's threshold; t=0 resets
                    # nothing (state is 0), s0=0 makes the op a plain add
                    rst = float(4.0 ** (t - 1)) if t > 0 else 0.0
                    rthr = _prevfloat(4.0 ** (t - 1)) if t > 0 else 1.0
                    src = pst[t % NSTATE]
                    dst = pst[(t + 1) % NSTATE]
                    ycol = (t - xoff) * F
                    kstop = 7 if g < G8 - 1 else 6
                    st = spool.tile([P, F], u8, name="s", tag="s")
                    s8 = st.bitcast(fp8)

                    if t < CHUNK_T0:
                        yt = xt[:, ycol : ycol + F]
                        if t == 0:
                            # P_0 = y_0 exactly: a 2x-mode copy replaces the
                            # state memset + full custom op
                            nc.vector.tensor_copy(dst, yt)
                        else:
                            nc.vector._custom_dve(
                                lif, out=dst, in0=src, in1=yt, s0=rst, s1=rthr
                            )
                        nc.scalar.activation(
                            st, dst, Act.Sign, bias=th[:, t : t + 1]
                        )
                        for c0 in range(0, F, Q):
                            nc.tensor.matmul(
                                acc[c0 // Q],
                                wt[:, 128 * k : 128 * (k + 1)],
                                s8[:, c0 : c0 + Q],
                                start=(k == 0),
                                stop=(k == kstop),
                            )
                        if k == 7 and g < G8 - 1:
                            pk = pkpool.tile([P, F], u8, name="pk", tag="pk")
                            for c0 in range(0, F, Q):
                                nc.scalar.activation(
                                    pk[:, c0 : c0 + Q], acc[c0 // Q],
                                    Act.Copy, scale=512.0,
                                )
                            nc.scalar.dma_start(out=o_d[g], in_=pk)
                    elif t < T - 1:
                        # steps 28-30: column-chunked recurrence + sign + mm
                        for c0 in range(0, F, Q):
                            sl = slice(c0, c0 + Q)
                            nc.vector._custom_dve(
                                lif,
                                out=dst[:, sl],
                                in0=src[:, sl],
                                in1=xt[:, ycol + c0 : ycol + c0 + Q],
                                s0=rst,
                                s1=rthr,
                            )
                            nc.scalar.activation(
                                st[:, sl], dst[:, sl], Act.Sign,
                                bias=th[:, t : t + 1],
                            )
                            nc.tensor.matmul(
                                acc[c0 // Q],
                                wt[:, 128 * k : 128 * (k + 1)],
                                s8[:, sl],
                                start=(k == 0),
                                stop=(k == kstop),
                            )
                        if t == T - 2:
                            # group 3's PSUM is final after step 30 (k=6);
                            # copy it out on DVE (scaled x512 -> integer u8)
                            # while DVE waits for step 31's quarter-loads.
                            # high_priority so the scheduler doesn't sink
                            # these behind step 31's recurrence chunks.
                            pk = pkpool.tile([P, F], u8, name="pk", tag="pk")
                            with tc.high_priority():
                                for c0 in range(0, F, Q):
                                    sl = slice(c0, c0 + Q)
                                    nc.vector.tensor_scalar_mul(
                                        pk[:, sl], acc[c0 // Q], 512.0
                                    )
                                    if c0 == Q:
                                        nc.scalar.dma_start(
                                            out=o_d[G8 - 1][:, : 2 * Q],
                                            in_=pk[:, : 2 * Q],
                                        )
                                    elif c0 == 3 * Q:
                                        nc.scalar.dma_start(
                                            out=o_d[G8 - 1][:, 2 * Q :],
                                            in_=pk[:, 2 * Q :],
                                        )
                    else:
                        # step 31: chunked recurrence + sign -> raw u8 store
                        for c0 in range(0, F, Q):
                            sl = slice(c0, c0 + Q)
                            nc.vector._custom_dve(
                                lif,
                                out=dst[:, sl],
                                in0=src[:, sl],
                                in1=xt[:, ycol + c0 : ycol + c0 + Q],
                                s0=rst,
                                s1=rthr,
                            )
                            nc.scalar.activation(
                                st[:, sl], dst[:, sl], Act.Sign,
                                bias=th[:, t : t + 1],
                            )
                            nc.scalar.dma_start(
                                out=o31_d[:, sl], in_=st[:, sl]
                            )
    nc.finalize()
    return nc


def _consts():
    import ml_dtypes

    w = np.zeros((P, 8 * 128), ml_dtypes.float8_e4m3)
    eye = np.eye(128)
    for k in range(8):
        w[:, 128 * k : 128 * (k + 1)] = (2.0**k * eye).astype(ml_dtypes.float8_e4m3)
    th = np.empty((P, T), np.float32)
    for t in range(T):
        th[:, t] = -np.float32(_prevfloat(4.0**t))
    return w, th


def kernel(x: np.ndarray) -> np.ndarray:
    from concourse.bass_utils import run_bass_kernel_spmd

    assert x.shape == (T, B, C, H, W) and x.dtype == np.float32
    if "nc" not in _cache:
        _cache["nc"] = _build()
    nc = _cache["nc"]

    # host-side pre-scale: y_t = 4^t * x_t (exact power-of-two multiply)
    scale = (4.0 ** np.arange(T, dtype=np.float64)).astype(np.float32)
    y = x * scale[:, None, None, None, None]
    w, th = _consts()

    in_maps = []
    for c in range(NCORES):
        yk = (
            y[:, c * BPC : (c + 1) * BPC]
            .reshape(T, P, F)
            .transpose(1, 0, 2)
            .reshape(P, T * F)
        )
        in_maps.append({"x": np.ascontiguousarray(yk), "w": w, "th": th})

    res = run_bass_kernel_spmd(nc, in_maps, core_ids=list(range(NCORES)))
    _cache["last_result"] = res

    # unpack: group g byte bit k (LSB-first) = spike at t = 8g+k for g<3;
    # group 3 bits 0-6 = steps 24-30; step 31 is raw u8 in o31
    bits = np.arange(8, dtype=np.uint8)
    out = np.empty((T, B, C, H, W), dtype=np.float32)
    for c in range(NCORES):
        pk = res.results[c]["o"]  # [G8, P, F] u8
        o31 = res.results[c]["o31"]  # [P, F] u8
        sp = (pk[:, None] >> bits[None, :, None, None]) & np.uint8(1)
        sp[G8 - 1, 7] = o31
        out[:, c * BPC : (c + 1) * BPC] = (
            sp.reshape(T, BPC, C, H, W).astype(np.float32)
        )
    return out
